# revision 38
# baseline (speedup 1.0000x reference)
"""Int4-packed linear (group-quantized, 256-group) on 8 Trainium2 cores.

Column-parallel: each core owns 1024 of 8192 out_features.

Math per core (out^T orientation, o on partitions):
  out[t, o] = sum_g s[o,g] * R_g[o,t] - 8*sum_g s[o,g]*xsum_g[t] + bias[o]
  R_g[o,t]  = sum_{i in g} q[o,i] * x[t,i]        (q in 0..15)

Weights ship as fp8e4m3 nibble planes (exact small integers), x as bf16.
Group partials accumulate in PSUM slices; -8 offset + bias ride a tiny fp32
correction matmul into group 31's slice (pre-divided by bf16(s[:,31]) so the
on-chip bf16 scale multiply restores it exactly).

Combine (per o-tile) is an ACT+DVE+GPSIMD pipeline in [g, t] layout.
During the startup dead time (weights still streaming in), ACT and DVE
pre-broadcast the scales into full [g, t] bf16 tiles; then steady-state:
  ACT    : drain PSUM fp32 -> SBUF bf16 (contiguous, releases PSUM fast)
  DVE    : sp = rsb * s_brc  (contiguous bf16 tensor_tensor, 2x mode)
  GPSIMD : half-tree level 1 (16->8 per half, bf16)
  DVE    : half-tree levels 2..4 + cross-half add
Weight DMA is split into 8 per-o-tile blocks across both HWDGE queues so
each o-tile's matmul chain starts as soon as its own block lands; y
accumulates in SBUF and ships as one 2KB-per-row DMA at the end.
"""

import sys

import numpy as np
import ml_dtypes

sys.path.insert(0, "/opt/trn_rl_repo")

import concourse.bass as bass  # noqa: E402
import concourse.mybir as mybir  # noqa: E402
import concourse.tile as tile  # noqa: E402
from concourse import bacc  # noqa: E402

NCORES = 8
TOKENS = 64
IN_F = 8192
OUT_F = 8192
GROUP = 256
OC = OUT_F // NCORES  # 1024 out-features per core
NCHUNK = IN_F // 128  # 64 K-chunks of 128
NG = IN_F // GROUP  # 32 groups
NOT = OC // 128  # 8 o-tiles per core

_cache = {}

ADD = mybir.AluOpType.add
MULT = mybir.AluOpType.mult


def _build_nc():
    if "nc" in _cache:
        return _cache["nc"], _cache["names"]

    f32 = mybir.dt.float32
    bf16 = mybir.dt.bfloat16
    fp16 = mybir.dt.float16
    fp8 = mybir.dt.float8e4
    NDR = 5  # tiles 0..NDR-1 use the ACT-drain + 2x mult path
    nc = bacc.Bacc(None, target_bir_lowering=False, debug=False)
    with tile.TileContext(nc) as tc:
        with tc.tile_pool(name="dram", bufs=1, space="DRAM") as dram:
            w8 = dram.tile([128, NOT, NCHUNK, 128], fp8, kind="ExternalInput")
            xt = dram.tile([128, NCHUNK, TOKENS], bf16, kind="ExternalInput")
            s2 = dram.tile([128, NOT, NG], fp16, kind="ExternalInput")
            cl = dram.tile([NG + 1, OC], bf16, kind="ExternalInput")
            cr = dram.tile([NG + 1, TOKENS], bf16, kind="ExternalInput")
            outT = dram.tile([128, NOT, TOKENS], f32, kind="ExternalOutput")

            with (
                tc.tile_pool(name="wsb", bufs=1) as wsb,
                tc.tile_pool(name="xsb", bufs=1) as xsb,
                tc.tile_pool(name="small", bufs=1) as small,
                tc.tile_pool(name="rsbp", bufs=2) as rsbp,
                tc.tile_pool(name="spp", bufs=2) as spp,
                tc.tile_pool(name="t16", bufs=2) as t16p,
                tc.tile_pool(name="t8", bufs=2) as t8p,
                tc.tile_pool(name="t4", bufs=2) as t4p,
                tc.tile_pool(name="t2", bufs=2) as t2p,
                tc.tile_pool(name="yout", bufs=1) as ypool,
                tc.tile_pool(name="ps", bufs=2, space="PSUM") as ps,
            ):
                w_all = wsb.tile([128, NOT, NCHUNK, 128], fp8)
                x_all = xsb.tile([128, NCHUNK, TOKENS], bf16)
                s2_all = small.tile([128, NOT, NG], fp16)
                cl_sb = small.tile([NG + 1, OC], bf16, tag="cl")
                cr_sb = small.tile([NG + 1, TOKENS], bf16, tag="cr")
                y_all = ypool.tile([128, NOT, TOKENS], f32)
                s_brc = small.tile([128, NDR, NG, TOKENS], fp16, tag="sbrc")

                # SP queue carries all the big tensors (it sustains
                # ~350GB/s solo, no mid-stream stalls); x split around w0
                # so chain 0 starts as early as possible.
                H2 = NCHUNK // 2
                nc.sync.dma_start(out=s2_all[:], in_=s2[:])
                nc.sync.dma_start(out=x_all[:, :H2, :], in_=xt[:, :H2, :])
                nc.sync.dma_start(out=w_all[:, 0, :H2, :], in_=w8[:, 0, :H2, :])
                nc.sync.dma_start(out=x_all[:, H2:, :], in_=xt[:, H2:, :])
                nc.sync.dma_start(out=w_all[:, 0, H2:, :], in_=w8[:, 0, H2:, :])
                nc.scalar.dma_start(out=cl_sb[:], in_=cl[:])
                nc.scalar.dma_start(out=cr_sb[:], in_=cr[:])
                for b in range(1, NOT):
                    nc.sync.dma_start(out=w_all[:, b, :, :], in_=w8[:, b, :, :])

                # ACT pre-broadcasts the scales during the startup dead
                # time; the last few are interleaved between early drains
                # (emitted inside the main loop) so drains take priority
                for ot in range(NDR):
                    s2_ot = s2_all[:, ot, :]
                    bc_ap = bass.AP(
                        tensor=s2_ot.tensor,
                        offset=s2_ot.offset,
                        ap=[s2_ot.ap[0], [1, NG], [0, TOKENS]],
                    )
                    nc.scalar.copy(out=s_brc[:, ot], in_=bc_ap)

                t16s, sps = {}, {}

                def chain(ot):
                    osl = slice(ot * 128, (ot + 1) * 128)
                    r_ps = ps.tile([128, NG, TOKENS], f32)
                    for g in range(NG):
                        nc.tensor.matmul(
                            r_ps[:, g, :],
                            lhsT=w_all[:, ot, 2 * g, :],
                            rhs=x_all[:, 2 * g, :],
                            start=True,
                            stop=False,
                        )
                        nc.tensor.matmul(
                            r_ps[:, g, :],
                            lhsT=w_all[:, ot, 2 * g + 1, :],
                            rhs=x_all[:, 2 * g + 1, :],
                            start=False,
                            stop=(g != NG - 1),
                        )
                    # -8 offset + bias correction, pre-divided by fp16(s31)
                    nc.tensor.matmul(
                        r_ps[:, NG - 1, :],
                        lhsT=cl_sb[:, osl],
                        rhs=cr_sb[:],
                        start=False,
                        stop=True,
                    )
                    sp = spp.tile([128, NG, TOKENS], fp16)
                    if ot < NDR:
                        # ACT drain -> DVE 2x mult against the broadcast
                        rsb = rsbp.tile([128, NG, TOKENS], fp16)
                        nc.scalar.copy(out=rsb[:], in_=r_ps[:])
                        nc.vector.tensor_tensor(
                            out=sp[:], in0=rsb[:], in1=s_brc[:, ot], op=MULT)
                    else:
                        # late tiles: DVE multiplies straight from PSUM (1x)
                        # with the stride-0 scales AP -- no broadcast needed
                        s2_ot = s2_all[:, ot, :]
                        s_ap = bass.AP(
                            tensor=s2_ot.tensor,
                            offset=s2_ot.offset,
                            ap=[s2_ot.ap[0], [1, NG], [0, TOKENS]],
                        )
                        nc.vector.tensor_tensor(
                            out=sp[:], in0=r_ps[:], in1=s_ap, op=MULT)
                    sps[ot] = sp

                def l1(ot):  # gpsimd level 1, 32 -> 16
                    sp = sps.pop(ot)
                    t16 = t16p.tile([128, 16, TOKENS], fp16)
                    nc.gpsimd.tensor_tensor(
                        out=t16[:], in0=sp[:, 0:16, :], in1=sp[:, 16:32, :],
                        op=ADD)
                    t16s[ot] = t16

                def tail(ot):  # DVE levels 2..5
                    t16 = t16s.pop(ot)
                    t8 = t8p.tile([128, 8, TOKENS], fp16)
                    nc.vector.tensor_tensor(
                        out=t8[:], in0=t16[:, 0:8, :], in1=t16[:, 8:16, :],
                        op=ADD)
                    t4 = t4p.tile([128, 4, TOKENS], fp16)
                    nc.vector.tensor_tensor(
                        out=t4[:], in0=t8[:, 0:4, :], in1=t8[:, 4:8, :],
                        op=ADD)
                    t2 = t2p.tile([128, 2, TOKENS], f32)
                    nc.vector.tensor_tensor(
                        out=t2[:], in0=t4[:, 0:2, :], in1=t4[:, 2:4, :],
                        op=ADD)
                    nc.vector.tensor_tensor(
                        out=y_all[:, ot, :], in0=t2[:, 0, :], in1=t2[:, 1, :],
                        op=ADD)

                for ot in range(NOT - 3):
                    chain(ot)
                    l1(ot)
                    if ot >= 1:
                        tail(ot - 1)

                # last three tiles run entirely on DVE: PSUM-direct mult,
                # DVE level-1 (2x), then the tree -- no cross-engine latency
                for ot in (NOT - 3, NOT - 2, NOT - 1):
                    chain(ot)
                    sp = sps.pop(ot)
                    t16 = t16p.tile([128, 16, TOKENS], fp16, tag=f"f{ot}")
                    nc.vector.tensor_tensor(
                        out=t16[:], in0=sp[:, 0:16, :], in1=sp[:, 16:32, :],
                        op=ADD)
                    t16s[ot] = t16
                    tail(ot)
                    if ot == NOT - 3:
                        tail(NOT - 4)

                nc.scalar.dma_start(out=outT[:], in_=y_all[:])

    nc.compile()
    names = dict(w8=w8.name, xt=xt.name, s2=s2.name, cl=cl.name, cr=cr.name,
                 outT=outT.name)
    _cache["nc"] = nc
    _cache["names"] = names
    return nc, names


def _host_prep(x, weight_packed, scales, bias):
    """Build the 8 per-core input maps."""
    _, names = _build_nc()

    bf16 = ml_dtypes.bfloat16
    wp = np.ascontiguousarray(weight_packed).view(np.uint32)  # [8192, 1024]
    shifts = (np.arange(8, dtype=np.uint32) * 4)[None, None, :]
    nib = ((wp[:, :, None] >> shifts) & np.uint32(0xF)).astype(np.uint8)
    nib = nib.reshape(OUT_F, IN_F)  # n[o, i]
    lut = np.arange(16, dtype=np.float32).astype(ml_dtypes.float8_e4m3)
    nfp8 = lut[nib]  # [8192, 8192] fp8, exact

    xb = x.astype(bf16)
    xf = xb.astype(np.float32)
    # xt_host[p, c, t] = x_bf16[t, 128c + p]
    xt_host = np.ascontiguousarray(xb.T.reshape(NCHUNK, 128, TOKENS).transpose(1, 0, 2))
    # xsum_g[t] (with bf16-rounded x, matching the matmul operand)
    xsum = xf.reshape(TOKENS, NG, GROUP).sum(axis=2)  # [t, g]
    cr_host = np.concatenate(
        [xsum.T, np.ones((1, TOKENS), dtype=np.float32)], axis=0
    ).astype(bf16)  # [33, 64] bf16

    in_maps = []
    for k in range(NCORES):
        osl = slice(OC * k, OC * (k + 1))
        nk = nfp8[osl]  # [1024, 8192]
        # w8_host[p, b, c, j] = n[128b + j, 128c + p]
        w8_host = np.ascontiguousarray(
            nk.reshape(NOT, 128, NCHUNK, 128).transpose(3, 0, 2, 1)
        )
        sck = np.asarray(scales[osl], dtype=np.float32)  # [1024, 32]
        sb = sck.astype(np.float16)  # fp16 scales used on-chip
        s31b = sb[:, NG - 1].astype(np.float32)  # fp16-rounded s31
        # s2_host[p, ot, g] = fp16(s[128*ot + p, g])
        s2_host = np.ascontiguousarray(
            sb.reshape(NOT, 128, NG).transpose(1, 0, 2)
        )  # [128, 8, 32] fp16
        cl_host = np.empty((NG + 1, OC), dtype=np.float32)
        cl_host[:NG] = (-8.0 * sck / s31b[:, None]).T
        cl_host[NG] = np.asarray(bias[osl], dtype=np.float32) / s31b
        cl_host = cl_host.astype(bf16)
        in_maps.append({
            names["w8"]: w8_host,
            names["xt"]: xt_host,
            names["s2"]: s2_host,
            names["cl"]: cl_host,
            names["cr"]: cr_host,
        })
    return in_maps


def kernel(x, weight_packed, scales, bias):
    from concourse.bass_utils import run_bass_kernel_spmd

    nc, names = _build_nc()
    in_maps = _host_prep(x, weight_packed, scales, bias)
    res = run_bass_kernel_spmd(nc, in_maps, core_ids=list(range(NCORES)))
    # outT[p, ot, t] -> out[t, k*1024 + ot*128 + p]
    outs = [
        np.asarray(res.results[k][names["outT"]]).transpose(1, 0, 2).reshape(OC, TOKENS)
        for k in range(NCORES)
    ]
    out = np.concatenate([o.T for o in outs], axis=1)  # [64, 8192]
    return np.ascontiguousarray(out.astype(np.float32))


# revision 40
# speedup vs baseline: 1.0060x; 1.0060x over previous
"""Int4-packed linear (group-quantized, 256-group) on 8 Trainium2 cores.

Column-parallel: each core owns 1024 of 8192 out_features.

Math per core (out^T orientation, o on partitions):
  out[t, o] = sum_g s[o,g] * R_g[o,t] - 8*sum_g s[o,g]*xsum_g[t] + bias[o]
  R_g[o,t]  = sum_{i in g} q[o,i] * x[t,i]        (q in 0..15)

Weights ship as fp8e4m3 nibble planes (exact small integers), x as bf16.
Group partials accumulate in PSUM slices; -8 offset + bias ride a tiny fp32
correction matmul into group 31's slice (pre-divided by bf16(s[:,31]) so the
on-chip bf16 scale multiply restores it exactly).

Combine (per o-tile) is an ACT+DVE+GPSIMD pipeline in [g, t] layout.
During the startup dead time (weights still streaming in), ACT and DVE
pre-broadcast the scales into full [g, t] bf16 tiles; then steady-state:
  ACT    : drain PSUM fp32 -> SBUF bf16 (contiguous, releases PSUM fast)
  DVE    : sp = rsb * s_brc  (contiguous bf16 tensor_tensor, 2x mode)
  GPSIMD : half-tree level 1 (16->8 per half, bf16)
  DVE    : half-tree levels 2..4 + cross-half add
Weight DMA is split into 8 per-o-tile blocks across both HWDGE queues so
each o-tile's matmul chain starts as soon as its own block lands; y
accumulates in SBUF and ships as one 2KB-per-row DMA at the end.
"""

import sys

import numpy as np
import ml_dtypes

sys.path.insert(0, "/opt/trn_rl_repo")

import concourse.bass as bass  # noqa: E402
import concourse.mybir as mybir  # noqa: E402
import concourse.tile as tile  # noqa: E402
from concourse import bacc  # noqa: E402

NCORES = 8
TOKENS = 64
IN_F = 8192
OUT_F = 8192
GROUP = 256
OC = OUT_F // NCORES  # 1024 out-features per core
NCHUNK = IN_F // 128  # 64 K-chunks of 128
NG = IN_F // GROUP  # 32 groups
NOT = OC // 128  # 8 o-tiles per core

_cache = {}

ADD = mybir.AluOpType.add
MULT = mybir.AluOpType.mult


def _build_nc():
    if "nc" in _cache:
        return _cache["nc"], _cache["names"]

    f32 = mybir.dt.float32
    bf16 = mybir.dt.bfloat16
    fp16 = mybir.dt.float16
    fp8 = mybir.dt.float8e4
    NDR = 5  # tiles 0..NDR-1 use the ACT-drain + 2x mult path
    nc = bacc.Bacc(None, target_bir_lowering=False, debug=False)
    with tile.TileContext(nc) as tc:
        with tc.tile_pool(name="dram", bufs=1, space="DRAM") as dram:
            w8 = dram.tile([128, NOT, NCHUNK, 128], fp8, kind="ExternalInput")
            xt = dram.tile([128, NCHUNK, TOKENS], bf16, kind="ExternalInput")
            s2 = dram.tile([128, NOT, NG], fp16, kind="ExternalInput")
            cl = dram.tile([NG + 1, OC], f32, kind="ExternalInput")
            cr = dram.tile([NG + 1, TOKENS], f32, kind="ExternalInput")
            outT = dram.tile([128, NOT, TOKENS], f32, kind="ExternalOutput")

            with (
                tc.tile_pool(name="wsb", bufs=1) as wsb,
                tc.tile_pool(name="xsb", bufs=1) as xsb,
                tc.tile_pool(name="small", bufs=1) as small,
                tc.tile_pool(name="rsbp", bufs=2) as rsbp,
                tc.tile_pool(name="spp", bufs=2) as spp,
                tc.tile_pool(name="t16", bufs=2) as t16p,
                tc.tile_pool(name="t8", bufs=2) as t8p,
                tc.tile_pool(name="t4", bufs=2) as t4p,
                tc.tile_pool(name="t2", bufs=2) as t2p,
                tc.tile_pool(name="yout", bufs=1) as ypool,
                tc.tile_pool(name="ps", bufs=2, space="PSUM") as ps,
            ):
                w_all = wsb.tile([128, NOT, NCHUNK, 128], fp8)
                x_all = xsb.tile([128, NCHUNK, TOKENS], bf16)
                s2_all = small.tile([128, NOT, NG], fp16)
                cl_sb = small.tile([NG + 1, OC], f32, tag="cl")
                cr_sb = small.tile([NG + 1, TOKENS], f32, tag="cr")
                y_all = ypool.tile([128, NOT, TOKENS], f32)
                s_brc = small.tile([128, NDR, NG, TOKENS], fp16, tag="sbrc")

                # SP queue carries all the big tensors (it sustains
                # ~350GB/s solo, no mid-stream stalls); x split around w0
                # so chain 0 starts as early as possible.
                nc.sync.dma_start(out=s2_all[:], in_=s2[:])
                nc.sync.dma_start(out=x_all[:, :NCHUNK // 2, :],
                                  in_=xt[:, :NCHUNK // 2, :])
                nc.sync.dma_start(out=w_all[:, 0, :, :], in_=w8[:, 0, :, :])
                # x's second half rides the near-empty Activation queue: it
                # lands ~11.5us (safely before that queue's busy window) and
                # sheds 0.5MB from the critical SP weight stream
                nc.scalar.dma_start(out=x_all[:, NCHUNK // 2:, :],
                                    in_=xt[:, NCHUNK // 2:, :])
                nc.scalar.dma_start(out=cl_sb[:], in_=cl[:])
                nc.scalar.dma_start(out=cr_sb[:], in_=cr[:])
                for b in range(1, NOT):
                    nc.sync.dma_start(out=w_all[:, b, :, :], in_=w8[:, b, :, :])

                # ACT pre-broadcasts the scales during the startup dead
                # time; the last few are interleaved between early drains
                # (emitted inside the main loop) so drains take priority
                for ot in range(NDR):
                    s2_ot = s2_all[:, ot, :]
                    bc_ap = bass.AP(
                        tensor=s2_ot.tensor,
                        offset=s2_ot.offset,
                        ap=[s2_ot.ap[0], [1, NG], [0, TOKENS]],
                    )
                    nc.scalar.copy(out=s_brc[:, ot], in_=bc_ap)

                t16s, sps = {}, {}

                def chain(ot):
                    osl = slice(ot * 128, (ot + 1) * 128)
                    r_ps = ps.tile([128, NG, TOKENS], f32)
                    for g in range(NG):
                        nc.tensor.matmul(
                            r_ps[:, g, :],
                            lhsT=w_all[:, ot, 2 * g, :],
                            rhs=x_all[:, 2 * g, :],
                            start=True,
                            stop=False,
                        )
                        nc.tensor.matmul(
                            r_ps[:, g, :],
                            lhsT=w_all[:, ot, 2 * g + 1, :],
                            rhs=x_all[:, 2 * g + 1, :],
                            start=False,
                            stop=(g != NG - 1),
                        )
                    # -8 offset + bias correction, pre-divided by fp16(s31)
                    nc.tensor.matmul(
                        r_ps[:, NG - 1, :],
                        lhsT=cl_sb[:, osl],
                        rhs=cr_sb[:],
                        start=False,
                        stop=True,
                    )
                    sp = spp.tile([128, NG, TOKENS], fp16)
                    if ot < NDR:
                        # ACT drain -> DVE 2x mult against the broadcast
                        rsb = rsbp.tile([128, NG, TOKENS], fp16)
                        nc.scalar.copy(out=rsb[:], in_=r_ps[:])
                        nc.vector.tensor_tensor(
                            out=sp[:], in0=rsb[:], in1=s_brc[:, ot], op=MULT)
                    else:
                        # late tiles: DVE multiplies straight from PSUM (1x)
                        # with the stride-0 scales AP -- no broadcast needed
                        s2_ot = s2_all[:, ot, :]
                        s_ap = bass.AP(
                            tensor=s2_ot.tensor,
                            offset=s2_ot.offset,
                            ap=[s2_ot.ap[0], [1, NG], [0, TOKENS]],
                        )
                        nc.vector.tensor_tensor(
                            out=sp[:], in0=r_ps[:], in1=s_ap, op=MULT)
                    sps[ot] = sp

                def l1(ot):  # gpsimd level 1, 32 -> 16
                    sp = sps.pop(ot)
                    t16 = t16p.tile([128, 16, TOKENS], fp16)
                    nc.gpsimd.tensor_tensor(
                        out=t16[:], in0=sp[:, 0:16, :], in1=sp[:, 16:32, :],
                        op=ADD)
                    t16s[ot] = t16

                def tail(ot):  # DVE levels 2..5
                    t16 = t16s.pop(ot)
                    t8 = t8p.tile([128, 8, TOKENS], fp16)
                    nc.vector.tensor_tensor(
                        out=t8[:], in0=t16[:, 0:8, :], in1=t16[:, 8:16, :],
                        op=ADD)
                    t4 = t4p.tile([128, 4, TOKENS], fp16)
                    nc.vector.tensor_tensor(
                        out=t4[:], in0=t8[:, 0:4, :], in1=t8[:, 4:8, :],
                        op=ADD)
                    t2 = t2p.tile([128, 2, TOKENS], f32)
                    nc.vector.tensor_tensor(
                        out=t2[:], in0=t4[:, 0:2, :], in1=t4[:, 2:4, :],
                        op=ADD)
                    nc.vector.tensor_tensor(
                        out=y_all[:, ot, :], in0=t2[:, 0, :], in1=t2[:, 1, :],
                        op=ADD)

                for ot in range(NOT - 3):
                    chain(ot)
                    l1(ot)
                    if ot >= 1:
                        tail(ot - 1)

                # last three tiles run entirely on DVE: PSUM-direct mult,
                # DVE level-1 (2x), then the tree -- no cross-engine latency
                for ot in (NOT - 3, NOT - 2, NOT - 1):
                    chain(ot)
                    sp = sps.pop(ot)
                    t16 = t16p.tile([128, 16, TOKENS], fp16, tag=f"f{ot}")
                    nc.vector.tensor_tensor(
                        out=t16[:], in0=sp[:, 0:16, :], in1=sp[:, 16:32, :],
                        op=ADD)
                    t16s[ot] = t16
                    tail(ot)
                    if ot == NOT - 3:
                        tail(NOT - 4)

                nc.scalar.dma_start(out=outT[:], in_=y_all[:])

    nc.compile()
    names = dict(w8=w8.name, xt=xt.name, s2=s2.name, cl=cl.name, cr=cr.name,
                 outT=outT.name)
    _cache["nc"] = nc
    _cache["names"] = names
    return nc, names


def _host_prep(x, weight_packed, scales, bias):
    """Build the 8 per-core input maps."""
    _, names = _build_nc()

    bf16 = ml_dtypes.bfloat16
    wp = np.ascontiguousarray(weight_packed).view(np.uint32)  # [8192, 1024]
    shifts = (np.arange(8, dtype=np.uint32) * 4)[None, None, :]
    nib = ((wp[:, :, None] >> shifts) & np.uint32(0xF)).astype(np.uint8)
    nib = nib.reshape(OUT_F, IN_F)  # n[o, i]
    lut = np.arange(16, dtype=np.float32).astype(ml_dtypes.float8_e4m3)
    nfp8 = lut[nib]  # [8192, 8192] fp8, exact

    xb = x.astype(bf16)
    xf = xb.astype(np.float32)
    # xt_host[p, c, t] = x_bf16[t, 128c + p]
    xt_host = np.ascontiguousarray(xb.T.reshape(NCHUNK, 128, TOKENS).transpose(1, 0, 2))
    # xsum_g[t] (with bf16-rounded x, matching the matmul operand)
    xsum = xf.reshape(TOKENS, NG, GROUP).sum(axis=2)  # [t, g]
    cr_host = np.concatenate(
        [xsum.T, np.ones((1, TOKENS), dtype=np.float32)], axis=0
    ).astype(np.float32)  # [33, 64]

    in_maps = []
    for k in range(NCORES):
        osl = slice(OC * k, OC * (k + 1))
        nk = nfp8[osl]  # [1024, 8192]
        # w8_host[p, b, c, j] = n[128b + j, 128c + p]
        w8_host = np.ascontiguousarray(
            nk.reshape(NOT, 128, NCHUNK, 128).transpose(3, 0, 2, 1)
        )
        sck = np.asarray(scales[osl], dtype=np.float32)  # [1024, 32]
        sb = sck.astype(np.float16)  # fp16 scales used on-chip
        s31b = sb[:, NG - 1].astype(np.float32)  # fp16-rounded s31
        # s2_host[p, ot, g] = fp16(s[128*ot + p, g])
        s2_host = np.ascontiguousarray(
            sb.reshape(NOT, 128, NG).transpose(1, 0, 2)
        )  # [128, 8, 32] fp16
        cl_host = np.empty((NG + 1, OC), dtype=np.float32)
        cl_host[:NG] = (-8.0 * sck / s31b[:, None]).T
        cl_host[NG] = np.asarray(bias[osl], dtype=np.float32) / s31b
        in_maps.append({
            names["w8"]: w8_host,
            names["xt"]: xt_host,
            names["s2"]: s2_host,
            names["cl"]: cl_host,
            names["cr"]: cr_host,
        })
    return in_maps


def kernel(x, weight_packed, scales, bias):
    from concourse.bass_utils import run_bass_kernel_spmd

    nc, names = _build_nc()
    in_maps = _host_prep(x, weight_packed, scales, bias)
    res = run_bass_kernel_spmd(nc, in_maps, core_ids=list(range(NCORES)))
    # outT[p, ot, t] -> out[t, k*1024 + ot*128 + p]
    outs = [
        np.asarray(res.results[k][names["outT"]]).transpose(1, 0, 2).reshape(OC, TOKENS)
        for k in range(NCORES)
    ]
    out = np.concatenate([o.T for o in outs], axis=1)  # [64, 8192]
    return np.ascontiguousarray(out.astype(np.float32))
